# revision 6
# baseline (speedup 1.0000x reference)
"""Trainium2 Bass kernel for nn_AdaConvNeXt (moe_routing) — v2.

Data-parallel over batch (16 images/core). Per image:
  - depthwise 7x7 conv entirely on TensorE: fp8 DoubleRow matmuls, 2 taps
    per pass (24 pairs + 1 single, even pair strides only), zero-padded
    34x36 fp8 input layout, PSUM accumulation across passes.
  - LN stats on the *gathered* token domain: y and y^2 interleaved
    (token-major [784, 6] layout), one gpsimd ap_gather per branch, then
    ones-vector matmuls; batched sqrt/reciprocal stat math per 2-image
    block; DRAM round-trip row broadcast.
  - Routing realized exactly: idx1/idx2 compaction via ap_gather; FFN and
    fast path run on 400-token compacted domains (fp8 DoubleRow matmuls,
    gelu on ScalarE); merge-back via one inverse-permutation ap_gather
    per output group (idx2-wins collision semantics preserved host-side).
  - Residual added in f32 (exact passthrough of x).
All matmul weights are host-folded (LN affine into w1/fp_w, gamma into
w2/fp_w, biases into gelu bias / epilogue biases) with dynamic power-of-2
fp8 scales.
"""

import os
import numpy as np
import ml_dtypes

import concourse.bass as bass
import concourse.bacc as bacc
import concourse.mybir as mybir
import concourse.tile as tile
from concourse.bass_utils import run_bass_kernel_spmd

VP = mybir._bass_rust.VecI64Pair
BF16 = mybir.dt.bfloat16
FP8 = mybir.dt.float8e4
F32 = mybir.dt.float32
I16 = mybir.dt.int16
ADD = mybir.AluOpType.add
MULT = mybir.AluOpType.mult
AF = mybir.ActivationFunctionType
DRM = mybir.MatmulPerfMode.DoubleRow

N_CORES = 8
B, C, H, W = 128, 384, 28, 28
N = H * W            # 784
BL = B // N_CORES    # 16 images per core
NG = C // 128        # 3 channel groups
FG = (4 * C) // 128  # 12 ffn groups
NT = 400             # padded gathered tokens per branch (392 -> 400)
NK = N // 2          # 392 real tokens per branch
EPS = 1e-6
HP, WP = 34, 36      # padded conv tile
PPITCH = NG * HP * WP  # per-partition elems of XPq

# ---- conv tap pairing (all 49 taps on PE; pair strides must be even) ----
def _tap_pairs():
    pairs = []
    for dy in range(-3, 4):
        pairs.append(((dy, -3), (dy, -1)))
        pairs.append(((dy, 1), (dy, 3)))
        pairs.append(((dy, -2), (dy, 0)))
    pairs.append(((-3, 2), (-2, 2)))
    pairs.append(((-1, 2), (0, 2)))
    pairs.append(((1, 2), (2, 2)))
    single = (3, 2)
    return pairs, single

PAIRS, SINGLE = _tap_pairs()
NPASS = len(PAIRS)  # 24


def _off(dy, dx):
    return (3 + dy) * WP + (3 + dx)


def cap(ap, aplist):
    c = ap.copy()
    c.ap = VP(aplist)
    return c


def build_bass(BL_, SD, S1, S2f, S2q):
    nc = bacc.Bacc(None, target_bir_lowering=False, debug=False)

    x_d = nc.declare_dram_parameter("x", [BL_, C, H, W], F32, isOutput=False)
    idx_d = nc.declare_dram_parameter("idxw", [BL_, 2, 128, 25], I16, isOutput=False)
    inv_d = nc.declare_dram_parameter("invw", [BL_, 128, 49], I16, isOutput=False)
    convdr_d = nc.declare_dram_parameter("convdr", [128, NG, NPASS, 2, 128], FP8, isOutput=False)
    convsg_d = nc.declare_dram_parameter("convsg", [128, NG, 128], FP8, isOutput=False)
    w1dr_d = nc.declare_dram_parameter("w1dr", [128, FG, 2, 128], FP8, isOutput=False)
    w1sg_d = nc.declare_dram_parameter("w1sg", [128, FG, 128], FP8, isOutput=False)
    w2fdr_d = nc.declare_dram_parameter("w2fdr", [128, NG, 6, 2, 128], FP8, isOutput=False)
    w2qdr_d = nc.declare_dram_parameter("w2qdr", [128, NG, 2, 128], FP8, isOutput=False)
    w2qsg_d = nc.declare_dram_parameter("w2qsg", [128, NG, 128], FP8, isOutput=False)
    # cvec cols: 0..2 dwb*SD, 3..14 c1, 15..17 c1out, 18..20 c2
    cvec_d = nc.declare_dram_parameter("cvec", [128, 21], F32, isOutput=False)
    out_d = nc.declare_dram_parameter("out", [BL_, C, H, W], F32, isOutput=True)

    from contextlib import ExitStack
    with ExitStack() as es:
        tc = es.enter_context(tile.TileContext(nc))
        pool = lambda name, bufs, **kw: es.enter_context(
            tc.tile_pool(name=name, bufs=bufs, **kw))
        cpool = pool("consts", 1)
        xpq_pool = pool("xpq", 2)
        y2x_pool = pool("y2x", 2)
        yg_pool = pool("yg", 2)
        tz_pool = pool("tz", 2)
        zq_pool = pool("zq", 2)
        gq_pool = pool("gq", 2)
        cat_pool = pool("cat", 2)
        og_pool = pool("og", 2)
        ox_pool = pool("ox", 2)
        xr_pool = pool("xr", 2)
        bg_pool = pool("bg", 4)
        idx_pool = pool("idx", 2)
        rows_pool = pool("rows", 1)
        dram_pool = pool("dscratch", 4, space=bass.MemorySpace.DRAM)
        py_pool = pool("py", 2, space=bass.MemorySpace.PSUM)
        pst_pool = pool("pst", 2, space=bass.MemorySpace.PSUM)
        ph_pool = pool("ph", 2, space=bass.MemorySpace.PSUM)
        pfq_pool = pool("pfq", 2, space=bass.MemorySpace.PSUM)

        # ---- constants ----
        convdr_sb = cpool.tile([128, NG, NPASS, 2, 128], FP8)
        for g in range(NG):
            nc.sync.dma_start(convdr_sb[:, g], convdr_d[:, g])
        convsg_sb = cpool.tile([128, NG, 128], FP8)
        nc.sync.dma_start(convsg_sb[:], convsg_d[:])
        w1dr_sb = cpool.tile([128, FG, 2, 128], FP8)
        nc.sync.dma_start(w1dr_sb[:], w1dr_d[:])
        w1sg_sb = cpool.tile([128, FG, 128], FP8)
        nc.sync.dma_start(w1sg_sb[:], w1sg_d[:])
        w2fdr_sb = cpool.tile([128, NG, 6, 2, 128], FP8)
        nc.sync.dma_start(w2fdr_sb[:], w2fdr_d[:])
        w2qdr_sb = cpool.tile([128, NG, 2, 128], FP8)
        nc.sync.dma_start(w2qdr_sb[:], w2qdr_d[:])
        w2qsg_sb = cpool.tile([128, NG, 128], FP8)
        nc.sync.dma_start(w2qsg_sb[:], w2qsg_d[:])
        cvec_sb = cpool.tile([128, 21], F32)
        nc.sync.dma_start(cvec_sb[:], cvec_d[:])
        ones_col = cpool.tile([128, 1], BF16)
        nc.vector.memset(ones_col[:], 1.0)
        eps_col = cpool.tile([33, 1], F32)
        nc.vector.memset(eps_col[:], float(SD) * float(SD) * EPS)

        n_blocks = (BL_ + 1) // 2
        for blk in range(n_blocks):
            imgs = list(range(blk * 2, min(blk * 2 + 2, BL_)))

            srow = rows_pool.tile([33, 2, NT], F32, tag="srow")
            qrow = rows_pool.tile([33, 2, NT], F32, tag="qrow")

            yg_t = {}
            idx_t = {}
            for ii, img in enumerate(imgs):
                ps = 32 * ii
                # ---- load padded fp8 input (SWDGE cast); zero borders on
                # first use of each of the 2 pool buffers ----
                xpq = xpq_pool.tile([128, NG, HP, WP], FP8)
                if blk == 0:
                    nc.vector.memset(xpq[:], 0.0)
                for g in range(NG):
                    nc.gpsimd.dma_start(
                        out=xpq[:, g, 3:31, 3:31],
                        in_=x_d[img, g * 128:(g + 1) * 128])

                idxw = idx_pool.tile([128, 2, 25], I16)
                nc.sync.dma_start(
                    out=idxw[:], in_=idx_d[img].rearrange("b p k -> p b k"))
                idx_t[img] = idxw

                # ---- depthwise conv on PE: fp8 DR pairs ----
                y2x = y2x_pool.tile([128, N, 6], BF16)
                for g in range(NG):
                    for h in range(2):
                        py = py_pool.tile([128, 14, W], F32, tag="py")
                        base = g * HP * WP + h * 14 * WP
                        for k, (ta, tb) in enumerate(PAIRS):
                            oa, ob_ = _off(*ta), _off(*tb)
                            rhs = cap(xpq[:, g, 0:14, 0:W],
                                      [[PPITCH, 128], [ob_ - oa, 2], [WP, 14], [1, W]])
                            rhs.offset = xpq[:].offset + base + oa
                            nc.tensor.matmul(
                                py[:], convdr_sb[:, g, k], rhs,
                                start=(k == 0), stop=False,
                                perf_mode=DRM, skip_group_check=True)
                        osg = _off(*SINGLE)
                        rhs1 = cap(xpq[:, g, 0:14, 0:W],
                                   [[PPITCH, 128], [WP, 14], [1, W]])
                        rhs1.offset = xpq[:].offset + base + osg
                        nc.tensor.matmul(
                            py[:], convsg_sb[:, g], rhs1,
                            start=False, stop=True, skip_group_check=True)
                        # merge: y_s = psum + SD*dw_b  (token-major strided out)
                        dst = cap(y2x[:], [[N * 6, 128], [6 * W, 14], [6, W]])
                        dst.offset = y2x[:].offset + h * 392 * 6 + 2 * g
                        nc.vector.tensor_scalar(
                            out=dst, in0=py[:], scalar1=cvec_sb[:, g:g + 1],
                            scalar2=None, op0=ADD)
                # ysq into odd slots (one op, 3 groups)
                sq_in = cap(y2x[:], [[N * 6, 128], [6, N], [2, NG]])
                sq_out = cap(y2x[:], [[N * 6, 128], [6, N], [2, NG]])
                sq_out.offset = y2x[:].offset + 1
                nc.vector.tensor_tensor(out=sq_out, in0=sq_in, in1=sq_in, op=MULT)

                # ---- gather per branch (y and y^2 together, d=6) ----
                yg = yg_pool.tile([128, 2, NT, 6], BF16)
                yg_t[img] = yg
                for br in range(2):
                    nc.gpsimd.ap_gather(
                        yg[:, br], y2x[:], idxw[:, br],
                        channels=128, num_elems=N, d=6, num_idxs=NT)

                # ---- stats via ones-matmuls on gathered domain ----
                for br in range(2):
                    pst = pst_pool.tile([33, NT], F32, tag="pst")
                    for g in range(NG):
                        rhs = cap(yg[:, br], [[2 * NT * 6, 128], [6, NT]])
                        rhs.offset = yg[:].offset + br * NT * 6 + 2 * g
                        nc.tensor.matmul(
                            pst[0:1, :], ones_col[:], rhs,
                            start=(g == 0), stop=(g == NG - 1),
                            skip_group_check=True)
                    for g in range(NG):
                        rhs = cap(yg[:, br], [[2 * NT * 6, 128], [6, NT]])
                        rhs.offset = yg[:].offset + br * NT * 6 + 2 * g + 1
                        nc.tensor.matmul(
                            pst[32:33, :], ones_col[:], rhs,
                            start=(g == 0), stop=(g == NG - 1),
                            tile_position=(0, 32), skip_group_check=True)
                    nc.scalar.activation(
                        srow[ps:ps + 1, br], pst[0:1, :], AF.Copy,
                        scale=1.0 / (SD * C))
                    nc.scalar.activation(
                        qrow[ps:ps + 1, br], pst[32:33, :], AF.Copy,
                        scale=1.0 / (SD * SD * C))

            # ---- batched stat math over the block ----
            np_ = 32 * (len(imgs) - 1) + 1
            musq = rows_pool.tile([33, 2, NT], F32, tag="musq")
            nc.vector.tensor_tensor(out=musq[:np_], in0=srow[:np_], in1=srow[:np_], op=MULT)
            veps = rows_pool.tile([33, 2, NT], F32, tag="veps")
            nc.vector.scalar_tensor_tensor(
                out=veps[:np_], in0=musq[:np_], scalar=-1.0, in1=qrow[:np_],
                op0=MULT, op1=ADD)
            sd_s = rows_pool.tile([33, 2, NT], F32, tag="musq")
            nc.scalar.activation(sd_s[:np_], veps[:np_], AF.Sqrt,
                                 bias=eps_col[:np_], scale=float(SD * SD))
            istd_r = rows_pool.tile([33, 2, NT], F32, tag="istd")
            with nc.allow_low_precision(reason="branch output is gamma-scaled"):
                nc.vector.reciprocal_approx_fast(out=istd_r[:np_], in_=sd_s[:np_])
            nmi_r = rows_pool.tile([33, 2, NT], F32, tag="veps")
            nc.vector.scalar_tensor_tensor(
                out=nmi_r[:np_], in0=srow[:np_], scalar=-float(SD), in1=istd_r[:np_],
                op0=MULT, op1=MULT)

            # stage rows in DRAM for partition-broadcast (bf16 cast on return)
            sc_t = {}
            for ii, img in enumerate(imgs):
                ps = 32 * ii
                sc = dram_pool.tile([2, 2 * NT], F32, tag="sc", name=f"sc{ii}")
                nc.sync.dma_start(out=sc[0:1, :], in_=istd_r[ps:ps + 1].rearrange("p a b -> p (a b)"))
                nc.sync.dma_start(out=sc[1:2, :], in_=nmi_r[ps:ps + 1].rearrange("p a b -> p (a b)"))
                sc_t[img] = sc

            # ---- phase 2 ----
            for ii, img in enumerate(imgs):
                yg = yg_t[img]
                sc = sc_t[img]
                idxw = idx_t[img]
                istd_bg = bg_pool.tile([128, 2, NT], BF16, tag="istdbg")
                nc.gpsimd.dma_start(
                    out=istd_bg[:], in_=sc[0:1, :].partition_broadcast(128))
                nmi_bg = bg_pool.tile([128, 2, NT], BF16, tag="nmibg")
                nc.gpsimd.dma_start(
                    out=nmi_bg[:], in_=sc[1:2, :].partition_broadcast(128))

                invw = idx_pool.tile([128, 49], I16, tag="invw")
                nc.sync.dma_start(out=invw[:], in_=inv_d[img])
                xres = xr_pool.tile([128, NG, N], F32)
                nc.scalar.dma_start(
                    out=xres[:], in_=x_d[img].rearrange("(g c) h w -> c g (h w)", g=NG))

                # ---- z (both branches, fp8) ----
                zq = zq_pool.tile([128, NG, 2 * NT], FP8)
                for g in range(NG):
                    tzt = tz_pool.tile([128, 2, NT], BF16, tag="tz")
                    src = cap(yg[:], [[2 * NT * 6, 128], [NT * 6, 2], [6, NT]])
                    src.offset = yg[:].offset + 2 * g
                    nc.vector.tensor_tensor(out=tzt[:], in0=src, in1=istd_bg[:], op=MULT)
                    nc.vector.tensor_tensor(
                        out=zq[:, g].rearrange("p (b t) -> p b t", b=2),
                        in0=tzt[:], in1=nmi_bg[:], op=ADD)

                # ---- FFN (branch 1, gathered) ----
                gq = gq_pool.tile([128, FG, NT], FP8)
                for fg in range(FG):
                    ph = ph_pool.tile([128, NT], F32, tag="ph")
                    rhs = cap(zq[:], [[NG * 2 * NT, 128], [2 * NT, 2], [1, NT]])
                    rhs.offset = zq[:].offset
                    nc.tensor.matmul(ph[:], w1dr_sb[:, fg], rhs,
                                     start=True, stop=False,
                                     perf_mode=DRM, skip_group_check=True)
                    nc.tensor.matmul(ph[:], w1sg_sb[:, fg], zq[:, 2, 0:NT],
                                     start=False, stop=True, skip_group_check=True)
                    nc.scalar.activation(
                        gq[:, fg], ph[:], AF.Gelu,
                        bias=cvec_sb[:, 3 + fg:4 + fg], scale=1.0 / S1)

                cat = cat_pool.tile([128, NG, 2 * NT], F32)
                outg = og_pool.tile([128, NG, N], F32)
                for og in range(NG):
                    pf = pfq_pool.tile([128, NT], F32, tag="pfq")
                    for j in range(6):
                        rhs = cap(gq[:], [[FG * NT, 128], [NT, 2], [1, NT]])
                        rhs.offset = gq[:].offset + 2 * j * NT
                        nc.tensor.matmul(pf[:], w2fdr_sb[:, og, j], rhs,
                                         start=(j == 0), stop=(j == 5),
                                         perf_mode=DRM, skip_group_check=True)
                    nc.scalar.activation(
                        cat[:, og, 0:NT], pf[:], AF.Identity,
                        bias=cvec_sb[:, 15 + og:16 + og], scale=1.0 / S2f)
                    pq = pfq_pool.tile([128, NT], F32, tag="pfq")
                    rhs = cap(zq[:], [[NG * 2 * NT, 128], [2 * NT, 2], [1, NT]])
                    rhs.offset = zq[:].offset + NT
                    nc.tensor.matmul(pq[:], w2qdr_sb[:, og], rhs,
                                     start=True, stop=False,
                                     perf_mode=DRM, skip_group_check=True)
                    nc.tensor.matmul(pq[:], w2qsg_sb[:, og], zq[:, 2, NT:2 * NT],
                                     start=False, stop=True, skip_group_check=True)
                    nc.scalar.activation(
                        cat[:, og, NT:2 * NT], pq[:], AF.Identity,
                        bias=cvec_sb[:, 18 + og:19 + og], scale=1.0 / S2q)
                    # zero-column for tokens in neither branch
                    nc.vector.memset(cat[:, og, 2 * NT - 4:2 * NT], 0.0)
                    nc.gpsimd.ap_gather(
                        outg[:, og], cat[:, og], invw[:],
                        channels=128, num_elems=2 * NT, d=1, num_idxs=N)

                ox = ox_pool.tile([128, NG, N], F32)
                nc.vector.tensor_tensor(out=ox[:], in0=outg[:], in1=xres[:], op=ADD)
                nc.sync.dma_start(
                    out=out_d[img].rearrange("(g c) h w -> c g (h w)", g=NG), in_=ox[:])
    nc.compile()
    return nc


# ---------------------------------------------------------------------------
# host side
# ---------------------------------------------------------------------------

def _pow2_scale(mat, target=64.0):
    m = float(np.abs(mat).max())
    if m == 0.0:
        return 1.0
    return float(2.0 ** np.floor(np.log2(target / m)))


def _fold_host(inputs):
    f32 = np.float32
    fp8 = ml_dtypes.float8_e4m3fn
    dw_w = np.asarray(inputs["dw_w"], f32)
    dw_b = np.asarray(inputs["dw_b"], f32)
    norm_w = np.asarray(inputs["norm_w"], f32)
    norm_b = np.asarray(inputs["norm_b"], f32)
    w1 = np.asarray(inputs["w1"], f32)
    b1 = np.asarray(inputs["b1"], f32)
    w2 = np.asarray(inputs["w2"], f32)
    b2 = np.asarray(inputs["b2"], f32)
    gamma = np.asarray(inputs["gamma"], f32)
    fp_norm_w = np.asarray(inputs["fp_norm_w"], f32)
    fp_norm_b = np.asarray(inputs["fp_norm_b"], f32)
    fp_w = np.asarray(inputs["fp_w"], f32)
    fp_b = np.asarray(inputs["fp_b"], f32)
    fp_gamma = np.asarray(inputs["fp_gamma"], f32)

    W1 = norm_w[:, None] * w1
    c1 = norm_b @ w1 + b1
    W2f = w2 * gamma[None, :]
    c1out = b2 * gamma
    W2q = (fp_norm_w[:, None] * fp_w) * fp_gamma[None, :]
    c2 = (fp_norm_b @ fp_w + fp_b) * fp_gamma

    SD = _pow2_scale(dw_w, 4.0)
    S1 = _pow2_scale(W1, 64.0)
    S2f = _pow2_scale(W2f, 64.0)
    S2q = _pow2_scale(W2q, 64.0)

    ar = np.arange(128)
    convdr = np.zeros((128, NG, NPASS, 2, 128), f32)
    convsg = np.zeros((128, NG, 128), f32)
    for g in range(NG):
        ch = slice(g * 128, (g + 1) * 128)
        for k, (ta, tb) in enumerate(PAIRS):
            convdr[ar, g, k, 0, ar] = dw_w[ch, 0, ta[0] + 3, ta[1] + 3] * SD
            convdr[ar, g, k, 1, ar] = dw_w[ch, 0, tb[0] + 3, tb[1] + 3] * SD
        convsg[ar, g, ar] = dw_w[ch, 0, SINGLE[0] + 3, SINGLE[1] + 3] * SD

    w1dr = np.zeros((128, FG, 2, 128), f32)
    w1sg = np.zeros((128, FG, 128), f32)
    for fg in range(FG):
        fs = slice(fg * 128, (fg + 1) * 128)
        w1dr[:, fg, 0] = W1[0:128, fs] * S1
        w1dr[:, fg, 1] = W1[128:256, fs] * S1
        w1sg[:, fg] = W1[256:384, fs] * S1
    w2fdr = np.zeros((128, NG, 6, 2, 128), f32)
    for og in range(NG):
        os_ = slice(og * 128, (og + 1) * 128)
        for j in range(6):
            w2fdr[:, og, j, 0] = W2f[(2 * j) * 128:(2 * j + 1) * 128, os_] * S2f
            w2fdr[:, og, j, 1] = W2f[(2 * j + 1) * 128:(2 * j + 2) * 128, os_] * S2f
    w2qdr = np.zeros((128, NG, 2, 128), f32)
    w2qsg = np.zeros((128, NG, 128), f32)
    for og in range(NG):
        os_ = slice(og * 128, (og + 1) * 128)
        w2qdr[:, og, 0] = W2q[0:128, os_] * S2q
        w2qdr[:, og, 1] = W2q[128:256, os_] * S2q
        w2qsg[:, og] = W2q[256:384, os_] * S2q

    cvec = np.zeros((128, 21), f32)
    for g in range(NG):
        cvec[:, g] = dw_b[g * 128:(g + 1) * 128] * SD
    for fg in range(FG):
        cvec[:, 3 + fg] = c1[fg * 128:(fg + 1) * 128]
    for og in range(NG):
        cvec[:, 15 + og] = c1out[og * 128:(og + 1) * 128]
        cvec[:, 18 + og] = c2[og * 128:(og + 1) * 128]

    return dict(
        convdr=convdr.astype(fp8), convsg=convsg.astype(fp8),
        w1dr=w1dr.astype(fp8), w1sg=w1sg.astype(fp8),
        w2fdr=w2fdr.astype(fp8), w2qdr=w2qdr.astype(fp8),
        w2qsg=w2qsg.astype(fp8), cvec=cvec,
    ), SD, S1, S2f, S2q


def _wrap16(vals, ncols):
    """Wrap a 1-D index list into the gpsimd [128, ncols] layout."""
    w = np.zeros((16, ncols), np.int16)
    for k, v in enumerate(vals):
        w[k % 16, k // 16] = v
    return np.tile(w, (8, 1))


def _indices_host(idx1, idx2, Bn):
    idxw = np.zeros((Bn, 2, 128, 25), np.int16)
    invw = np.zeros((Bn, 128, 49), np.int16)
    for i in range(Bn):
        i1 = np.asarray(idx1[i], np.int64)
        i2 = np.asarray(idx2[i], np.int64)
        s1 = np.concatenate([i1, np.zeros(NT - len(i1), np.int64)])
        s2 = np.concatenate([i2, np.zeros(NT - len(i2), np.int64)])
        idxw[i, 0] = _wrap16(s1, 25)
        idxw[i, 1] = _wrap16(s2, 25)
        inv = np.full(N, 2 * NT - 1, np.int64)  # default: zero column
        inv[i1] = np.arange(len(i1))
        inv[i2] = NT + np.arange(len(i2))       # idx2 wins collisions
        invw[i] = _wrap16(inv, 49)
    return idxw, invw


LAST_RESULT = None


def kernel(**inputs):
    global LAST_RESULT
    x = np.ascontiguousarray(np.asarray(inputs["x"], np.float32))
    Bn = x.shape[0]
    bl = Bn // N_CORES
    assert Bn % N_CORES == 0

    folded, SD, S1, S2f, S2q = _fold_host(inputs)
    idxw, invw = _indices_host(inputs["idx1"], inputs["idx2"], Bn)

    nc = build_bass(bl, SD, S1, S2f, S2q)

    in_maps = []
    for c in range(N_CORES):
        sl = slice(c * bl, (c + 1) * bl)
        in_maps.append(dict(
            x=x[sl], idxw=idxw[sl], invw=invw[sl], **folded))

    trace = bool(int(os.environ.get("BASS_KERNEL_TRACE", "0")))
    res = run_bass_kernel_spmd(nc, in_maps, list(range(N_CORES)), trace=trace)
    LAST_RESULT = res
    out = np.concatenate([res.results[c]["out"] for c in range(N_CORES)], axis=0)
    return out
